# revision 2
# baseline (speedup 1.0000x reference)
"""DynamicKVCache.update kernel for Trainium2 (8 NeuronCores).

Appends one new token's key/value onto the [B, L, H, D] K/V caches along the
sequence dim and returns the full [B, L+1, H, D] caches — pure memory
movement.

Sharding: data parallel over the batch dim (B=8 -> 1 batch element per core).
Per core the concat is a contiguous layout: new_k.flat = [cache_k.flat |
key.flat], so the kernel is four DRAM->DRAM DMA copies per core. The two
64 MiB cache copies are issued on the two independent HWDGE rings (sync/SP
and scalar/ACT) so the 16 SDMA engines round-robin between the two streams
at packet granularity — this overlaps HBM reads of one stream with HBM
writes of the other and runs at ~336 GB/s of HBM traffic per core (~94% of
the ~358 GB/s per-NC HBM limit), vs ~218 GB/s when both copies share one
ring.
"""
import numpy as np

import concourse.bass as bass
import concourse.mybir as mybir
from concourse.bass_utils import run_bass_kernel_spmd

# Problem shape (hardcoded; kernel.py must be self-contained).
B, L, T, H, D = 8, 4096, 1, 32, 128
CACHE = L * H * D          # 16,777,216 f32 elems = 64 MiB per batch element
NEW = T * H * D            # 4,096 f32 elems = 16 KiB
OUT = CACHE + NEW
N_CORES = 8
F32 = mybir.dt.float32

_NC = None


def _build():
    """One-round concat program: 4 DRAM->DRAM DMAs split across 2 HWDGE rings."""
    nc = bass.Bass()
    ck = nc.declare_dram_parameter("cache_k", [CACHE], F32, isOutput=False)
    cv = nc.declare_dram_parameter("cache_v", [CACHE], F32, isOutput=False)
    kk = nc.declare_dram_parameter("key", [NEW], F32, isOutput=False)
    vv = nc.declare_dram_parameter("value", [NEW], F32, isOutput=False)
    nk = nc.declare_dram_parameter("new_k", [OUT], F32, isOutput=True)
    nv = nc.declare_dram_parameter("new_v", [OUT], F32, isOutput=True)

    with nc.Block() as block, nc.semaphore("sem_k") as sk, nc.semaphore("sem_v") as sv:
        # NEFF completion requires every engine to reach its end, so each
        # engine only needs to await its own DMAs — no cross-engine waits.
        @block.sync
        def _(sync):
            sync.dma_start(out=nk[0:CACHE], in_=ck[:]).then_inc(sk, 16)
            sync.dma_start(out=nk[CACHE:OUT], in_=kk[:]).then_inc(sk, 16)
            sync.wait_ge(sk, 32)

        @block.scalar
        def _(scalar):
            scalar.dma_start(out=nv[0:CACHE], in_=cv[:]).then_inc(sv, 16)
            scalar.dma_start(out=nv[CACHE:OUT], in_=vv[:]).then_inc(sv, 16)
            scalar.wait_ge(sv, 32)
    return nc


def _get_nc():
    global _NC
    if _NC is None:
        _NC = _build()
    return _NC


def kernel(cache_k, cache_v, key, value):
    cache_k = np.ascontiguousarray(np.asarray(cache_k), dtype=np.float32)
    cache_v = np.ascontiguousarray(np.asarray(cache_v), dtype=np.float32)
    key = np.ascontiguousarray(np.asarray(key), dtype=np.float32)
    value = np.ascontiguousarray(np.asarray(value), dtype=np.float32)
    assert cache_k.shape == (B, L, H, D), cache_k.shape
    assert key.shape == (B, T, H, D), key.shape

    # Shard over batch: core i owns batch element i (flat per-core views).
    in_maps = [
        {
            "cache_k": cache_k[i].reshape(CACHE),
            "cache_v": cache_v[i].reshape(CACHE),
            "key": key[i].reshape(NEW),
            "value": value[i].reshape(NEW),
        }
        for i in range(N_CORES)
    ]

    res = run_bass_kernel_spmd(_get_nc(), in_maps, list(range(N_CORES)))

    # Gather: stack per-core outputs back to [B, L+T, H, D].
    new_k = np.stack([res.results[i]["new_k"].reshape(L + T, H, D) for i in range(N_CORES)])
    new_v = np.stack([res.results[i]["new_v"].reshape(L + T, H, D) for i in range(N_CORES)])
    return new_k, new_v
